# revision 5
# baseline (speedup 1.0000x reference)
"""Trainium2 Bass kernel for nn_MHA_36584531427723.

Sharding: 8 cores = 2 batches x 4 head-groups (4 heads of 64 dims each per
core). Each core computes its batch's Q/K/V projections restricted to its
head-group's 256 output features, attention for its 4 heads, and a partial
output projection (its 256 rows of Wo^T). The host sums the 4 partials per
batch and adds bo.

v2 (vs baseline):
  - Q/K projections and QK^T scores run as fp8e4m3 DoubleRow matmuls
    (host ships Q^T/K^T and Wq/Wk in fp8; K^T additionally in bf16 for the
    V projection). End-to-end rel err ~5e-3 (numpy study), gate is 2e-2.
  - Q_^T/K_^T are stored [128p = 4 heads x 32 feats, 2 slots, 2048] fp8 so
    per-head DoubleRow slices are [32, 2, *] with tile_position (32h, 0).
  - Phase B software-pipelines one (head, qg) unit: scores of unit i+1
    issue before PV of unit i, so the PE never stalls on Act's exp stream.
  - Mask multiplies split DVE/Pool (Pool via scalar_tensor_tensor which is
    cheaper than tensor_tensor on GPSIMD); s_row copy on DVE, not Act.
  - softmax: max-subtraction dropped (|E/32| < ~0.6); eps dropped.
"""

import numpy as np
import ml_dtypes

import concourse.bacc as bacc
import concourse.bass as bass  # noqa: F401
import concourse.mybir as mybir
import concourse.tile as tile
from concourse.bass_utils import run_bass_kernel_spmd

B, N, D = 2, 2048, 1024
H = 16
HD = 64
HL = 4  # heads per core
DL = HL * HD  # 256 local features
P = 128
KO = D // P  # 8 contraction chunks for projections
KOP = KO // 2  # 4 DoubleRow pair-chunks
NKC = N // P  # 16 k-token chunks
NQC = N // P
NPAN = 4
PANW = N // NPAN  # 512-wide token panels in the projection phase
SCALE = 1.0 / 32.0  # 1/sqrt(DIM_V)

# elementwise work split (per (h, qg) unit of 16 kc tiles)
MASK_DVE = 10  # kc < MASK_DVE: mask multiply on DVE; rest on Pool

F32 = mybir.dt.float32
BF16 = mybir.dt.bfloat16
FP8 = mybir.dt.float8e4
AF = mybir.ActivationFunctionType
DR = mybir.MatmulPerfMode.DoubleRow
ALU = mybir.AluOpType


def build_nc():
    nc = bacc.Bacc(None, target_bir_lowering=False)
    QT8 = nc.dram_tensor("qt8", (D, N), FP8, kind="ExternalInput")
    KT8 = nc.dram_tensor("kt8", (D, N), FP8, kind="ExternalInput")
    KT = nc.dram_tensor("kt", (D, N), BF16, kind="ExternalInput")
    MT = nc.dram_tensor("mt", (N, N), BF16, kind="ExternalInput")
    WQ8 = nc.dram_tensor("wq8", (P, KOP, 2, DL), FP8, kind="ExternalInput")
    WK8 = nc.dram_tensor("wk8", (P, KOP, 2, DL), FP8, kind="ExternalInput")
    WVT = nc.dram_tensor("wvt", (D, DL), BF16, kind="ExternalInput")
    WOT = nc.dram_tensor("wot", (DL, D), BF16, kind="ExternalInput")
    BQ = nc.dram_tensor("bq", (P, 2), F32, kind="ExternalInput")
    BK = nc.dram_tensor("bk", (P, 2), F32, kind="ExternalInput")
    BV = nc.dram_tensor("bv", (DL,), F32, kind="ExternalInput")
    OUT = nc.dram_tensor("out", (N, D), F32, kind="ExternalOutput")

    qt8_r = QT8[:].rearrange("(ko p) q -> p ko q", p=P)
    kt8_r = KT8[:].rearrange("(ko p) q -> p ko q", p=P)
    kt_r = KT[:].rearrange("(ko p) q -> p ko q", p=P)
    mt_r = MT[:].rearrange("(kc p) q -> p kc q", p=P)

    with tile.TileContext(nc) as tc:
        with (
            tc.tile_pool(name="persist", bufs=1) as persist,
            tc.tile_pool(name="otpool", bufs=1) as otpool,
        ):
            # --- persistent tiles ---
            mT = persist.tile([P, NKC, N], BF16)  # 64KB/part
            qT8 = persist.tile([P, 2, N], FP8, tag="qT8")  # 4KB
            kT8 = persist.tile([P, 2, N], FP8, tag="kT8")
            v_sb = persist.tile([P, NKC, HL, HD + 1], BF16, tag="v")
            ones_sb = persist.tile([1, HD], F32, tag="ones")
            nc.vector.memset(ones_sb[:], 1.0)
            bq_sb = persist.tile([P, 2], F32, tag="bq")
            bk_sb = persist.tile([P, 2], F32, tag="bk")
            bv_rep = persist.tile([P, HL, HD], F32, tag="bv")
            wo_sb = persist.tile([P, 2, D], BF16, tag="wo")

            nc.sync.dma_start(out=bq_sb[:], in_=BQ[:])
            nc.sync.dma_start(out=bk_sb[:], in_=BK[:])
            nc.sync.dma_start(
                out=bv_rep[:],
                in_=BV[:].rearrange("(h d) -> h d", h=HL)[None].to_broadcast(
                    (P, HL, HD)
                ),
            )
            for cc in range(2):
                nc.sync.dma_start(
                    out=wo_sb[:, cc, :],
                    in_=WOT[:].rearrange("(cc p) n -> p cc n", p=P)[:, cc, :],
                )
            nc.vector.memset(v_sb[:, :, :, HD : HD + 1], 1.0)

            # ---------------- Phase A: projections ----------------
            with (
                tc.tile_pool(name="wpool", bufs=1) as wpool,
                tc.tile_pool(name="panpool", bufs=2) as panpool,
                tc.tile_pool(name="pjpsum", bufs=4, space="PSUM") as pjpsum,
                tc.tile_pool(name="vpsum", bufs=4, space="PSUM") as vpsum,
            ):
                wq_sb = wpool.tile([P, KOP, 2, DL], FP8, tag="wq")
                wk_sb = wpool.tile([P, KOP, 2, DL], FP8, tag="wk")
                wv_sb = wpool.tile([P, KO, DL], BF16, tag="wv")
                nc.sync.dma_start(out=wq_sb[:], in_=WQ8[:])
                nc.sync.dma_start(out=wk_sb[:], in_=WK8[:])
                nc.sync.dma_start(
                    out=wv_sb[:], in_=WVT[:].rearrange("(ko p) m -> p ko m", p=P)
                )

                for pan in range(NPAN):
                    qs = slice(pan * PANW, (pan + 1) * PANW)
                    qt8_pan = panpool.tile([P, KO, PANW], FP8, tag="qt8_pan")
                    kt8_pan = panpool.tile([P, KO, PANW], FP8, tag="kt8_pan")
                    kt_pan = panpool.tile([P, KO, PANW], BF16, tag="kt_pan")
                    for ko in range(KO):
                        nc.sync.dma_start(out=kt8_pan[:, ko, :], in_=kt8_r[:, ko, qs])
                        nc.sync.dma_start(out=kt_pan[:, ko, :], in_=kt_r[:, ko, qs])
                        nc.sync.dma_start(out=qt8_pan[:, ko, :], in_=qt8_r[:, ko, qs])

                    # K_^T then Q_^T via fp8 DoubleRow; bias fused in evict
                    for pan_in, w_sb, b_sb, dst in (
                        (kt8_pan, wk_sb, bk_sb, kT8),
                        (qt8_pan, wq_sb, bq_sb, qT8),
                    ):
                        for dc in range(2):
                            ps = pjpsum.tile([P, PANW], F32, tag="pj")
                            for j in range(KOP):
                                nc.tensor.matmul(
                                    ps[:],
                                    lhsT=w_sb[:, j, :, dc * P : (dc + 1) * P],
                                    rhs=pan_in[:, 2 * j : 2 * j + 2, :],
                                    start=(j == 0),
                                    stop=(j == KOP - 1),
                                    perf_mode=DR,
                                )
                            nc.scalar.activation(
                                out=dst[:, dc, qs],
                                in_=ps[:],
                                func=AF.Identity,
                                bias=b_sb[:, dc : dc + 1],
                                scale=1.0,
                            )

                    # V natural layout (token-on-partition), bias via DVE add
                    for t4 in range(PANW // P):
                        tci = pan * (PANW // P) + t4
                        psv = vpsum.tile([P, DL], F32, tag="pv")
                        for ko in range(KO):
                            nc.tensor.matmul(
                                psv[:],
                                lhsT=kt_pan[:, ko, t4 * P : (t4 + 1) * P],
                                rhs=wv_sb[:, ko, :],
                                start=(ko == 0),
                                stop=(ko == KO - 1),
                            )
                        nc.vector.tensor_add(
                            out=v_sb[:, tci, :, 0:HD],
                            in0=psv[:].rearrange("p (h d) -> p h d", h=HL),
                            in1=bv_rep[:],
                        )

                # mask load last so it fills DMA gaps during phase A
                for kc in range(NKC):
                    nc.sync.dma_start(out=mT[:, kc, :], in_=mt_r[:, kc, :])

            # ---------------- Phase B: attention ----------------
            oT = otpool.tile([P, 2, N], BF16)
            with (
                tc.tile_pool(name="expool", bufs=2) as expool,
                tc.tile_pool(name="srpool", bufs=2) as srpool,
                tc.tile_pool(name="spsum", bufs=2, space="PSUM") as spsum,
                tc.tile_pool(name="opsum", bufs=2, space="PSUM") as opsum,
            ):

                def scores_block(h, qg):
                    """16 kc tiles of exp(mask-scaled scores) for unit (h,qg)."""
                    ex = expool.tile([P, NKC, 1024], BF16, tag="ex")
                    hp = slice(32 * h, 32 * h + 32)
                    for kc in range(NKC):
                        ps = spsum.tile([P, 1024], F32, tag="es")
                        for half in range(2):
                            q0 = qg * 1024 + half * 512
                            nc.tensor.matmul(
                                ps[:, half * 512 : (half + 1) * 512],
                                lhsT=kT8[hp, :, kc * P : (kc + 1) * P],
                                rhs=qT8[hp, :, q0 : q0 + 512],
                                start=True,
                                stop=True,
                                perf_mode=DR,
                                tile_position=(32 * h, 0),
                            )
                        nc.scalar.activation(
                            out=ex[:, kc, :], in_=ps[:], func=AF.Exp, scale=SCALE
                        )
                        mslice = mT[:, kc, qg * 1024 : (qg + 1) * 1024]
                        if kc < MASK_DVE:
                            nc.vector.tensor_mul(
                                out=ex[:, kc, :], in0=ex[:, kc, :], in1=mslice
                            )
                        else:
                            nc.gpsimd.tensor_mul(
                                out=ex[:, kc, :], in0=ex[:, kc, :], in1=mslice
                            )
                    return ex

                def pv_block(h, qg, ex):
                    dc, po = h // 2, (h % 2) * HD
                    for qbh in range(2):
                        pso = opsum.tile([HD + 1, 512], F32, tag="pvo")
                        for kc in range(NKC):
                            nc.tensor.matmul(
                                pso[:],
                                lhsT=v_sb[:, kc, h, :],
                                rhs=ex[:, kc, qbh * 512 : (qbh + 1) * 512],
                                start=(kc == 0),
                                stop=(kc == NKC - 1),
                            )
                        s_row = srpool.tile([1, 512], F32, tag="srow")
                        nc.vector.tensor_copy(out=s_row[:], in_=pso[HD : HD + 1, :])
                        srp = opsum.tile([HD, 512], F32, tag="srp")
                        nc.tensor.matmul(
                            srp[:],
                            lhsT=ones_sb[:],
                            rhs=s_row[:],
                            start=True,
                            stop=True,
                        )
                        s_rep = srpool.tile([HD, 512], F32, tag="srep")
                        nc.vector.reciprocal(out=s_rep[:], in_=srp[:])
                        o_tmp = srpool.tile([HD, 512], BF16, tag="otmp")
                        nc.vector.tensor_mul(
                            out=o_tmp[:], in0=pso[0:HD, :], in1=s_rep[:]
                        )
                        q0 = qg * 1024 + qbh * 512
                        nc.sync.dma_start(
                            out=oT[po : po + HD, dc, q0 : q0 + 512], in_=o_tmp[:]
                        )

                units = [(h, qg) for h in range(HL) for qg in range(2)]
                prev = None
                for h, qg in units:
                    ex = scores_block(h, qg)
                    if prev is not None:
                        pv_block(*prev)
                    prev = (h, qg, ex)
                pv_block(*prev)

            # ---------------- Phase C: output projection ----------------
            with (
                tc.tile_pool(name="cout", bufs=3) as cout,
                tc.tile_pool(name="cpsum", bufs=3, space="PSUM") as cpsum,
            ):
                for qc in range(NQC):
                    pss = cpsum.tile([P, D], F32, tag="co")
                    for cc in range(2):
                        for nh in range(2):
                            nc.tensor.matmul(
                                pss[:, nh * 512 : (nh + 1) * 512],
                                lhsT=oT[:, cc, qc * P : (qc + 1) * P],
                                rhs=wo_sb[:, cc, nh * 512 : (nh + 1) * 512],
                                start=(cc == 0),
                                stop=(cc == 1),
                            )
                    o_sb = cout.tile([P, D], F32, tag="osb")
                    nc.vector.tensor_copy(out=o_sb[:], in_=pss[:])
                    nc.sync.dma_start(out=OUT[qc * P : (qc + 1) * P, :], in_=o_sb[:])

    nc.finalize()
    return nc


_NC = None


def _get_nc():
    global _NC
    if _NC is None:
        _NC = build_nc()
    return _NC


def _dr_weight(W, cols):
    """[P, KOP, 2, DL] fp8 layout for the DoubleRow projection of W[cols,:].T.

    Output feature order: slot s (dc), partition p -> local feature
    h*64 + s*32 + (p%32) with h = p//32.
    """
    Wl = W[cols, :]  # [DL, D] rows = local features
    p_idx = np.arange(P)
    lf = (p_idx // 32) * 64 + (p_idx % 32)  # base feature per partition
    out = np.empty((P, KOP, 2, DL), np.float32)
    for s in range(2):
        # [D, 128] input-feature-major for this slot
        w_slot = Wl[lf + 32 * s, :].T  # [D, 128]
        out[:, :, :, s * P : (s + 1) * P] = w_slot.reshape(KOP, 2, P, P).transpose(
            2, 0, 1, 3
        )
    return np.ascontiguousarray(out).astype(ml_dtypes.float8_e4m3)


def _dr_bias(b, cols):
    bl = np.asarray(b, np.float32)[cols]
    p_idx = np.arange(P)
    lf = (p_idx // 32) * 64 + (p_idx % 32)
    out = np.stack([bl[lf], bl[lf + 32]], axis=1)  # [P, 2]
    return np.ascontiguousarray(out)


def make_in_maps(Q, K, mask, Wq, bq, Wk, bk, Wv, bv, Wo, bo):
    Q = np.asarray(Q, np.float32)
    K = np.asarray(K, np.float32)
    mask = np.asarray(mask)
    Wq = np.asarray(Wq, np.float32)
    Wk = np.asarray(Wk, np.float32)
    Wv = np.asarray(Wv, np.float32)
    Wo = np.asarray(Wo, np.float32)
    qt8 = [
        np.ascontiguousarray(Q[b].T).astype(ml_dtypes.float8_e4m3) for b in range(B)
    ]
    kt8 = [
        np.ascontiguousarray(K[b].T).astype(ml_dtypes.float8_e4m3) for b in range(B)
    ]
    kt = [np.ascontiguousarray(K[b].T).astype(ml_dtypes.bfloat16) for b in range(B)]
    mt = [
        np.ascontiguousarray(mask[b].T).astype(ml_dtypes.bfloat16) for b in range(B)
    ]
    in_maps = []
    for c in range(8):
        b, hg = divmod(c, 4)
        cols = slice(hg * DL, (hg + 1) * DL)
        in_maps.append(
            {
                "qt8": qt8[b],
                "kt8": kt8[b],
                "kt": kt[b],
                "mt": mt[b],
                "wq8": _dr_weight(Wq, cols),
                "wk8": _dr_weight(Wk, cols),
                "wvt": np.ascontiguousarray(Wv[cols, :].T).astype(ml_dtypes.bfloat16),
                "wot": np.ascontiguousarray(Wo[:, cols].T).astype(ml_dtypes.bfloat16),
                "bq": _dr_bias(bq, cols),
                "bk": _dr_bias(bk, cols),
                "bv": np.ascontiguousarray(np.asarray(bv, np.float32)[cols]),
            }
        )
    return in_maps


def assemble(results, bo):
    O = np.zeros((B, N, D), np.float32)
    for c in range(8):
        b = c // 4
        O[b] += results[c]["out"]
    O += np.asarray(bo, np.float32)[None, None, :]
    return O


def kernel(Q, K, mask, Wq, bq, Wk, bk, Wv, bv, Wo, bo):
    nc = _get_nc()
    in_maps = make_in_maps(Q, K, mask, Wq, bq, Wk, bk, Wv, bv, Wo, bo)
    res = run_bass_kernel_spmd(nc, in_maps, core_ids=list(range(8)))
    return assemble(res.results, bo)


# revision 6
# speedup vs baseline: 1.0195x; 1.0195x over previous
"""Trainium2 Bass kernel for nn_MHA_36584531427723.

Sharding: 8 cores = 2 batches x 4 head-groups (4 heads of 64 dims each per
core). Each core computes its batch's Q/K/V projections restricted to its
head-group's 256 output features, attention for its 4 heads, and a partial
output projection (its 256 rows of Wo^T). The host sums the 4 partials per
batch and adds bo.

v2 (vs baseline):
  - Q/K projections and QK^T scores run as fp8e4m3 DoubleRow matmuls
    (host ships Q^T/K^T and Wq/Wk in fp8; K^T additionally in bf16 for the
    V projection). End-to-end rel err ~5e-3 (numpy study), gate is 2e-2.
  - Q_^T/K_^T are stored [128p = 4 heads x 32 feats, 2 slots, 2048] fp8 so
    per-head DoubleRow slices are [32, 2, *] with tile_position (32h, 0).
  - Phase B software-pipelines one (head, qg) unit: scores of unit i+1
    issue before PV of unit i, so the PE never stalls on Act's exp stream.
  - Mask multiplies split DVE/Pool (Pool via scalar_tensor_tensor which is
    cheaper than tensor_tensor on GPSIMD); s_row copy on DVE, not Act.
  - softmax: max-subtraction dropped (|E/32| < ~0.6); eps dropped.
"""

import numpy as np
import ml_dtypes

import concourse.bacc as bacc
import concourse.bass as bass  # noqa: F401
import concourse.mybir as mybir
import concourse.tile as tile
from concourse.bass_utils import run_bass_kernel_spmd

B, N, D = 2, 2048, 1024
H = 16
HD = 64
HL = 4  # heads per core
DL = HL * HD  # 256 local features
P = 128
KO = D // P  # 8 contraction chunks for projections
KOP = KO // 2  # 4 DoubleRow pair-chunks
NKC = N // P  # 16 k-token chunks
NQC = N // P
NPAN = 4
PANW = N // NPAN  # 512-wide token panels in the projection phase
SCALE = 1.0 / 32.0  # 1/sqrt(DIM_V)

# elementwise work split (per (h, qg) unit of 16 kc tiles)
MASK_DVE = 10  # kc < MASK_DVE: mask multiply on DVE; rest on Pool

F32 = mybir.dt.float32
BF16 = mybir.dt.bfloat16
FP8 = mybir.dt.float8e4
AF = mybir.ActivationFunctionType
DR = mybir.MatmulPerfMode.DoubleRow
ALU = mybir.AluOpType


def build_nc():
    nc = bacc.Bacc(None, target_bir_lowering=False)
    QT8 = nc.dram_tensor("qt8", (D, N), FP8, kind="ExternalInput")
    KT8 = nc.dram_tensor("kt8", (D, N), FP8, kind="ExternalInput")
    KT = nc.dram_tensor("kt", (D, N), BF16, kind="ExternalInput")
    MT = nc.dram_tensor("mt", (N, N), BF16, kind="ExternalInput")
    WQ8 = nc.dram_tensor("wq8", (P, KOP, 2, DL), FP8, kind="ExternalInput")
    WK8 = nc.dram_tensor("wk8", (P, KOP, 2, DL), FP8, kind="ExternalInput")
    WVT = nc.dram_tensor("wvt", (D, DL), BF16, kind="ExternalInput")
    WOT = nc.dram_tensor("wot", (DL, D), BF16, kind="ExternalInput")
    BQ = nc.dram_tensor("bq", (P, 2), F32, kind="ExternalInput")
    BK = nc.dram_tensor("bk", (P, 2), F32, kind="ExternalInput")
    BV = nc.dram_tensor("bv", (DL,), F32, kind="ExternalInput")
    OUT = nc.dram_tensor("out", (N, D), F32, kind="ExternalOutput")

    qt8_r = QT8[:].rearrange("(ko p) q -> p ko q", p=P)
    kt8_r = KT8[:].rearrange("(ko p) q -> p ko q", p=P)
    kt_r = KT[:].rearrange("(ko p) q -> p ko q", p=P)
    mt_r = MT[:].rearrange("(kc p) q -> p kc q", p=P)

    with tile.TileContext(nc) as tc:
        with (
            tc.tile_pool(name="persist", bufs=1) as persist,
            tc.tile_pool(name="otpool", bufs=1) as otpool,
        ):
            # --- persistent tiles ---
            mT = persist.tile([P, NKC, N], BF16)  # 64KB/part
            qT8 = persist.tile([P, 2, N], FP8, tag="qT8")  # 4KB
            kT8 = persist.tile([P, 2, N], FP8, tag="kT8")
            v_sb = persist.tile([P, NKC, HL, HD + 1], BF16, tag="v")
            ones_sb = persist.tile([1, HD], F32, tag="ones")
            nc.vector.memset(ones_sb[:], 1.0)
            bq_sb = persist.tile([P, 2], F32, tag="bq")
            bk_sb = persist.tile([P, 2], F32, tag="bk")
            bv_rep = persist.tile([P, HL, HD], F32, tag="bv")
            wo_sb = persist.tile([P, 2, D], BF16, tag="wo")

            nc.sync.dma_start(out=bq_sb[:], in_=BQ[:])
            nc.sync.dma_start(out=bk_sb[:], in_=BK[:])
            nc.sync.dma_start(
                out=bv_rep[:],
                in_=BV[:].rearrange("(h d) -> h d", h=HL)[None].to_broadcast(
                    (P, HL, HD)
                ),
            )
            for cc in range(2):
                nc.sync.dma_start(
                    out=wo_sb[:, cc, :],
                    in_=WOT[:].rearrange("(cc p) n -> p cc n", p=P)[:, cc, :],
                )
            nc.vector.memset(v_sb[:, :, :, HD : HD + 1], 1.0)

            # ---------------- Phase A: projections ----------------
            with (
                tc.tile_pool(name="wpool", bufs=1) as wpool,
                tc.tile_pool(name="panpool", bufs=2) as panpool,
                tc.tile_pool(name="pjpsum", bufs=4, space="PSUM") as pjpsum,
                tc.tile_pool(name="vpsum", bufs=4, space="PSUM") as vpsum,
            ):
                wq_sb = wpool.tile([P, KOP, 2, DL], FP8, tag="wq")
                wk_sb = wpool.tile([P, KOP, 2, DL], FP8, tag="wk")
                wv_sb = wpool.tile([P, KO, DL], BF16, tag="wv")
                nc.sync.dma_start(out=wq_sb[:], in_=WQ8[:])
                nc.sync.dma_start(out=wk_sb[:], in_=WK8[:])
                nc.sync.dma_start(
                    out=wv_sb[:], in_=WVT[:].rearrange("(ko p) m -> p ko m", p=P)
                )

                for pan in range(NPAN):
                    qs = slice(pan * PANW, (pan + 1) * PANW)
                    qt8_pan = panpool.tile([P, KO, PANW], FP8, tag="qt8_pan")
                    kt8_pan = panpool.tile([P, KO, PANW], FP8, tag="kt8_pan")
                    kt_pan = panpool.tile([P, KO, PANW], BF16, tag="kt_pan")
                    for ko in range(KO):
                        nc.sync.dma_start(out=kt8_pan[:, ko, :], in_=kt8_r[:, ko, qs])
                        nc.sync.dma_start(out=kt_pan[:, ko, :], in_=kt_r[:, ko, qs])
                        nc.sync.dma_start(out=qt8_pan[:, ko, :], in_=qt8_r[:, ko, qs])

                    # K_^T then Q_^T via fp8 DoubleRow; bias fused in evict
                    for pan_in, w_sb, b_sb, dst in (
                        (kt8_pan, wk_sb, bk_sb, kT8),
                        (qt8_pan, wq_sb, bq_sb, qT8),
                    ):
                        for dc in range(2):
                            ps = pjpsum.tile([P, PANW], F32, tag="pj")
                            for j in range(KOP):
                                nc.tensor.matmul(
                                    ps[:],
                                    lhsT=w_sb[:, j, :, dc * P : (dc + 1) * P],
                                    rhs=pan_in[:, 2 * j : 2 * j + 2, :],
                                    start=(j == 0),
                                    stop=(j == KOP - 1),
                                    perf_mode=DR,
                                )
                            nc.scalar.activation(
                                out=dst[:, dc, qs],
                                in_=ps[:],
                                func=AF.Identity,
                                bias=b_sb[:, dc : dc + 1],
                                scale=1.0,
                            )

                    # V natural layout (token-on-partition), bias via DVE add
                    for t4 in range(PANW // P):
                        tci = pan * (PANW // P) + t4
                        psv = vpsum.tile([P, DL], F32, tag="pv")
                        for ko in range(KO):
                            nc.tensor.matmul(
                                psv[:],
                                lhsT=kt_pan[:, ko, t4 * P : (t4 + 1) * P],
                                rhs=wv_sb[:, ko, :],
                                start=(ko == 0),
                                stop=(ko == KO - 1),
                            )
                        nc.vector.tensor_add(
                            out=v_sb[:, tci, :, 0:HD],
                            in0=psv[:].rearrange("p (h d) -> p h d", h=HL),
                            in1=bv_rep[:],
                        )

                # mask load last so it fills DMA gaps during phase A
                for kc in range(NKC):
                    nc.sync.dma_start(out=mT[:, kc, :], in_=mt_r[:, kc, :])

            # ---------------- Phase B: attention ----------------
            oT = otpool.tile([P, 2, N], BF16)
            with (
                tc.tile_pool(name="expool", bufs=2) as expool,
                tc.tile_pool(name="srpool", bufs=2) as srpool,
                tc.tile_pool(name="spsum", bufs=2, space="PSUM") as spsum,
                tc.tile_pool(name="opsum", bufs=2, space="PSUM") as opsum,
            ):

                def scores_block(h, qg):
                    """16 kc tiles of exp(mask-scaled scores) for unit (h,qg)."""
                    ex = expool.tile([P, NKC, 1024], BF16, tag="ex")
                    hp = slice(32 * h, 32 * h + 32)
                    for kc in range(NKC):
                        ps = spsum.tile([P, 1024], F32, tag="es")
                        for half in range(2):
                            q0 = qg * 1024 + half * 512
                            nc.tensor.matmul(
                                ps[:, half * 512 : (half + 1) * 512],
                                lhsT=kT8[hp, :, kc * P : (kc + 1) * P],
                                rhs=qT8[hp, :, q0 : q0 + 512],
                                start=True,
                                stop=True,
                                perf_mode=DR,
                                tile_position=(32 * h, 0),
                            )
                        nc.scalar.activation(
                            out=ex[:, kc, :], in_=ps[:], func=AF.Exp, scale=SCALE
                        )
                        mslice = mT[:, kc, qg * 1024 : (qg + 1) * 1024]
                        # slow Pool masks on EARLY kc so they hide under the
                        # Act exp stream; fast DVE masks take the unit tail
                        if kc >= NKC - MASK_DVE:
                            nc.vector.tensor_mul(
                                out=ex[:, kc, :], in0=ex[:, kc, :], in1=mslice
                            )
                        else:
                            nc.gpsimd.tensor_mul(
                                out=ex[:, kc, :], in0=ex[:, kc, :], in1=mslice
                            )
                    return ex

                def pv_block(h, qg, ex):
                    dc, po = h // 2, (h % 2) * HD
                    for qbh in range(2):
                        pso = opsum.tile([HD + 1, 512], F32, tag="pvo")
                        for kc in range(NKC):
                            nc.tensor.matmul(
                                pso[:],
                                lhsT=v_sb[:, kc, h, :],
                                rhs=ex[:, kc, qbh * 512 : (qbh + 1) * 512],
                                start=(kc == 0),
                                stop=(kc == NKC - 1),
                            )
                        s_row = srpool.tile([1, 512], F32, tag="srow")
                        nc.vector.tensor_copy(out=s_row[:], in_=pso[HD : HD + 1, :])
                        srp = opsum.tile([HD, 512], F32, tag="srp")
                        nc.tensor.matmul(
                            srp[:],
                            lhsT=ones_sb[:],
                            rhs=s_row[:],
                            start=True,
                            stop=True,
                        )
                        s_rep = srpool.tile([HD, 512], F32, tag="srep")
                        nc.vector.reciprocal(out=s_rep[:], in_=srp[:])
                        o_tmp = srpool.tile([HD, 512], BF16, tag="otmp")
                        nc.vector.tensor_mul(
                            out=o_tmp[:], in0=pso[0:HD, :], in1=s_rep[:]
                        )
                        q0 = qg * 1024 + qbh * 512
                        nc.sync.dma_start(
                            out=oT[po : po + HD, dc, q0 : q0 + 512], in_=o_tmp[:]
                        )

                units = [(h, qg) for h in range(HL) for qg in range(2)]
                prev = None
                for h, qg in units:
                    ex = scores_block(h, qg)
                    if prev is not None:
                        pv_block(*prev)
                    prev = (h, qg, ex)
                pv_block(*prev)

            # ---------------- Phase C: output projection ----------------
            with (
                tc.tile_pool(name="cout", bufs=3) as cout,
                tc.tile_pool(name="cpsum", bufs=3, space="PSUM") as cpsum,
            ):
                for qc in range(NQC):
                    pss = cpsum.tile([P, D], F32, tag="co")
                    for cc in range(2):
                        for nh in range(2):
                            nc.tensor.matmul(
                                pss[:, nh * 512 : (nh + 1) * 512],
                                lhsT=oT[:, cc, qc * P : (qc + 1) * P],
                                rhs=wo_sb[:, cc, nh * 512 : (nh + 1) * 512],
                                start=(cc == 0),
                                stop=(cc == 1),
                            )
                    o_sb = cout.tile([P, D], F32, tag="osb")
                    nc.vector.tensor_copy(out=o_sb[:], in_=pss[:])
                    nc.sync.dma_start(out=OUT[qc * P : (qc + 1) * P, :], in_=o_sb[:])

    nc.finalize()
    return nc


_NC = None


def _get_nc():
    global _NC
    if _NC is None:
        _NC = build_nc()
    return _NC


def _dr_weight(W, cols):
    """[P, KOP, 2, DL] fp8 layout for the DoubleRow projection of W[cols,:].T.

    Output feature order: slot s (dc), partition p -> local feature
    h*64 + s*32 + (p%32) with h = p//32.
    """
    Wl = W[cols, :]  # [DL, D] rows = local features
    p_idx = np.arange(P)
    lf = (p_idx // 32) * 64 + (p_idx % 32)  # base feature per partition
    out = np.empty((P, KOP, 2, DL), np.float32)
    for s in range(2):
        # [D, 128] input-feature-major for this slot
        w_slot = Wl[lf + 32 * s, :].T  # [D, 128]
        out[:, :, :, s * P : (s + 1) * P] = w_slot.reshape(KOP, 2, P, P).transpose(
            2, 0, 1, 3
        )
    return np.ascontiguousarray(out).astype(ml_dtypes.float8_e4m3)


def _dr_bias(b, cols):
    bl = np.asarray(b, np.float32)[cols]
    p_idx = np.arange(P)
    lf = (p_idx // 32) * 64 + (p_idx % 32)
    out = np.stack([bl[lf], bl[lf + 32]], axis=1)  # [P, 2]
    return np.ascontiguousarray(out)


def make_in_maps(Q, K, mask, Wq, bq, Wk, bk, Wv, bv, Wo, bo):
    Q = np.asarray(Q, np.float32)
    K = np.asarray(K, np.float32)
    mask = np.asarray(mask)
    Wq = np.asarray(Wq, np.float32)
    Wk = np.asarray(Wk, np.float32)
    Wv = np.asarray(Wv, np.float32)
    Wo = np.asarray(Wo, np.float32)
    qt8 = [
        np.ascontiguousarray(Q[b].T).astype(ml_dtypes.float8_e4m3) for b in range(B)
    ]
    kt8 = [
        np.ascontiguousarray(K[b].T).astype(ml_dtypes.float8_e4m3) for b in range(B)
    ]
    kt = [np.ascontiguousarray(K[b].T).astype(ml_dtypes.bfloat16) for b in range(B)]
    mt = [
        np.ascontiguousarray(mask[b].T).astype(ml_dtypes.bfloat16) for b in range(B)
    ]
    in_maps = []
    for c in range(8):
        b, hg = divmod(c, 4)
        cols = slice(hg * DL, (hg + 1) * DL)
        in_maps.append(
            {
                "qt8": qt8[b],
                "kt8": kt8[b],
                "kt": kt[b],
                "mt": mt[b],
                "wq8": _dr_weight(Wq, cols),
                "wk8": _dr_weight(Wk, cols),
                "wvt": np.ascontiguousarray(Wv[cols, :].T).astype(ml_dtypes.bfloat16),
                "wot": np.ascontiguousarray(Wo[:, cols].T).astype(ml_dtypes.bfloat16),
                "bq": _dr_bias(bq, cols),
                "bk": _dr_bias(bk, cols),
                "bv": np.ascontiguousarray(np.asarray(bv, np.float32)[cols]),
            }
        )
    return in_maps


def assemble(results, bo):
    O = np.zeros((B, N, D), np.float32)
    for c in range(8):
        b = c // 4
        O[b] += results[c]["out"]
    O += np.asarray(bo, np.float32)[None, None, :]
    return O


def kernel(Q, K, mask, Wq, bq, Wk, bk, Wv, bv, Wo, bo):
    nc = _get_nc()
    in_maps = make_in_maps(Q, K, mask, Wq, bq, Wk, bk, Wv, bv, Wo, bo)
    res = run_bass_kernel_spmd(nc, in_maps, core_ids=list(range(8)))
    return assemble(res.results, bo)


# revision 9
# speedup vs baseline: 1.2086x; 1.1855x over previous
"""Trainium2 Bass kernel for nn_MHA_36584531427723.

Sharding: 8 cores = 2 batches x 4 head-groups (4 heads of 64 dims each per
core). Each core computes its batch's Q/K/V projections restricted to its
head-group's 256 output features, attention for its 4 heads, and a partial
output projection (its 256 rows of Wo^T). The host sums the 4 partials per
batch and adds bo.

v2 (vs baseline):
  - Q/K projections and QK^T scores run as fp8e4m3 DoubleRow matmuls
    (host ships Q^T/K^T and Wq/Wk in fp8; K^T additionally in bf16 for the
    V projection). End-to-end rel err ~5e-3 (numpy study), gate is 2e-2.
  - Q_^T/K_^T are stored [128p = 4 heads x 32 feats, 2 slots, 2048] fp8 so
    per-head DoubleRow slices are [32, 2, *] with tile_position (32h, 0).
  - Phase B software-pipelines one (head, qg) unit: scores of unit i+1
    issue before PV of unit i, so the PE never stalls on Act's exp stream.
  - Mask multiplies split DVE/Pool (Pool via scalar_tensor_tensor which is
    cheaper than tensor_tensor on GPSIMD); s_row copy on DVE, not Act.
  - softmax: max-subtraction dropped (|E/32| < ~0.6); eps dropped.
"""

import numpy as np
import ml_dtypes

import concourse.bacc as bacc
import concourse.bass as bass  # noqa: F401
import concourse.mybir as mybir
import concourse.tile as tile
from concourse.bass_utils import run_bass_kernel_spmd

B, N, D = 2, 2048, 1024
H = 16
HD = 64
HL = 4  # heads per core
DL = HL * HD  # 256 local features
P = 128
KO = D // P  # 8 contraction chunks for projections
KOP = KO // 2  # 4 DoubleRow pair-chunks
NKC = N // P  # 16 k-token chunks
NQC = N // P
NPAN = 4
PANW = N // NPAN  # 512-wide token panels in the projection phase
SCALE = 1.0 / 32.0  # 1/sqrt(DIM_V)

# elementwise work split (per (h, qg) unit of 16 kc tiles)
MASK_DVE = 10  # kc < MASK_DVE: mask multiply on DVE; rest on Pool

F32 = mybir.dt.float32
BF16 = mybir.dt.bfloat16
FP8 = mybir.dt.float8e4
AF = mybir.ActivationFunctionType
DR = mybir.MatmulPerfMode.DoubleRow
ALU = mybir.AluOpType


def build_nc():
    nc = bacc.Bacc(None, target_bir_lowering=False)
    QT8 = nc.dram_tensor("qt8", (D, N), FP8, kind="ExternalInput")
    KT8 = nc.dram_tensor("kt8", (D, N), FP8, kind="ExternalInput")
    KT = nc.dram_tensor("kt", (D, N), BF16, kind="ExternalInput")
    MT = nc.dram_tensor("mt", (N, N), BF16, kind="ExternalInput")
    WQ8 = nc.dram_tensor("wq8", (P, KOP, 2, DL), FP8, kind="ExternalInput")
    WK8 = nc.dram_tensor("wk8", (P, KOP, 2, DL), FP8, kind="ExternalInput")
    WVT = nc.dram_tensor("wvt", (D, DL), BF16, kind="ExternalInput")
    WOT = nc.dram_tensor("wot", (DL, D), BF16, kind="ExternalInput")
    BQ = nc.dram_tensor("bq", (P, 2), F32, kind="ExternalInput")
    BK = nc.dram_tensor("bk", (P, 2), F32, kind="ExternalInput")
    BV = nc.dram_tensor("bv", (DL,), F32, kind="ExternalInput")
    OUT = nc.dram_tensor("out", (N, D), F32, kind="ExternalOutput")

    qt8_r = QT8[:].rearrange("(ko p) q -> p ko q", p=P)
    kt8_r = KT8[:].rearrange("(ko p) q -> p ko q", p=P)
    kt_r = KT[:].rearrange("(ko p) q -> p ko q", p=P)
    mt_r = MT[:].rearrange("(kc p) q -> p kc q", p=P)

    with tile.TileContext(nc) as tc:
        with (
            tc.tile_pool(name="persist", bufs=1) as persist,
            tc.tile_pool(name="otpool", bufs=1) as otpool,
        ):
            # --- persistent tiles ---
            mT = persist.tile([P, NKC, N], BF16)  # 64KB/part
            qT8 = persist.tile([P, 2, N], FP8, tag="qT8")  # 4KB
            kT8 = persist.tile([P, 2, N], FP8, tag="kT8")
            v_sb = persist.tile([P, NKC, HL, 2 * HD], BF16, tag="v")
            bq_sb = persist.tile([P, 2], F32, tag="bq")
            bk_sb = persist.tile([P, 2], F32, tag="bk")
            bv_rep = persist.tile([P, HL, HD], F32, tag="bv")
            wo_sb = persist.tile([P, 2, D], BF16, tag="wo")

            nc.sync.dma_start(out=bq_sb[:], in_=BQ[:])
            nc.sync.dma_start(out=bk_sb[:], in_=BK[:])
            nc.sync.dma_start(
                out=bv_rep[:],
                in_=BV[:].rearrange("(h d) -> h d", h=HL)[None].to_broadcast(
                    (P, HL, HD)
                ),
            )
            for cc in range(2):
                nc.sync.dma_start(
                    out=wo_sb[:, cc, :],
                    in_=WOT[:].rearrange("(cc p) n -> p cc n", p=P)[:, cc, :],
                )
            nc.vector.memset(v_sb[:, :, :, HD:], 1.0)

            # ---------------- Phase A: projections ----------------
            with (
                tc.tile_pool(name="wpool", bufs=1) as wpool,
                tc.tile_pool(name="panpool", bufs=2) as panpool,
                tc.tile_pool(name="pjpsum", bufs=4, space="PSUM") as pjpsum,
                tc.tile_pool(name="vpsum", bufs=4, space="PSUM") as vpsum,
            ):
                wq_sb = wpool.tile([P, KOP, 2, DL], FP8, tag="wq")
                wk_sb = wpool.tile([P, KOP, 2, DL], FP8, tag="wk")
                wv_sb = wpool.tile([P, KO, DL], BF16, tag="wv")
                nc.sync.dma_start(out=wq_sb[:], in_=WQ8[:])
                nc.sync.dma_start(out=wk_sb[:], in_=WK8[:])
                nc.sync.dma_start(
                    out=wv_sb[:], in_=WVT[:].rearrange("(ko p) m -> p ko m", p=P)
                )

                for pan in range(NPAN):
                    qs = slice(pan * PANW, (pan + 1) * PANW)
                    qt8_pan = panpool.tile([P, KO, PANW], FP8, tag="qt8_pan")
                    kt8_pan = panpool.tile([P, KO, PANW], FP8, tag="kt8_pan")
                    kt_pan = panpool.tile([P, KO, PANW], BF16, tag="kt_pan")
                    for ko in range(KO):
                        nc.sync.dma_start(out=kt8_pan[:, ko, :], in_=kt8_r[:, ko, qs])
                        nc.sync.dma_start(out=kt_pan[:, ko, :], in_=kt_r[:, ko, qs])
                        nc.sync.dma_start(out=qt8_pan[:, ko, :], in_=qt8_r[:, ko, qs])

                    # K_^T then Q_^T via fp8 DoubleRow; bias fused in evict
                    for pan_in, w_sb, b_sb, dst in (
                        (kt8_pan, wk_sb, bk_sb, kT8),
                        (qt8_pan, wq_sb, bq_sb, qT8),
                    ):
                        for dc in range(2):
                            ps = pjpsum.tile([P, PANW], F32, tag="pj")
                            for j in range(KOP):
                                nc.tensor.matmul(
                                    ps[:],
                                    lhsT=w_sb[:, j, :, dc * P : (dc + 1) * P],
                                    rhs=pan_in[:, 2 * j : 2 * j + 2, :],
                                    start=(j == 0),
                                    stop=(j == KOP - 1),
                                    perf_mode=DR,
                                )
                            nc.scalar.activation(
                                out=dst[:, dc, qs],
                                in_=ps[:],
                                func=AF.Identity,
                                bias=b_sb[:, dc : dc + 1],
                                scale=1.0,
                            )

                    # V natural layout (token-on-partition), bias via DVE add
                    for t4 in range(PANW // P):
                        tci = pan * (PANW // P) + t4
                        psv = vpsum.tile([P, DL], F32, tag="pv")
                        for ko in range(KO):
                            nc.tensor.matmul(
                                psv[:],
                                lhsT=kt_pan[:, ko, t4 * P : (t4 + 1) * P],
                                rhs=wv_sb[:, ko, :],
                                start=(ko == 0),
                                stop=(ko == KO - 1),
                            )
                        nc.vector.tensor_add(
                            out=v_sb[:, tci, :, 0:HD],
                            in0=psv[:].rearrange("p (h d) -> p h d", h=HL),
                            in1=bv_rep[:],
                        )

                # mask load last so it fills DMA gaps during phase A
                for kc in range(NKC):
                    nc.sync.dma_start(out=mT[:, kc, :], in_=mt_r[:, kc, :])

            # ---------------- Phase B: attention ----------------
            # Lag-2 software pipeline: unit i's scores/exp/mask stream is
            # interleaved per-kc with unit (i-2)'s PV accumulation steps whose
            # inputs are all long since ready, so the in-order PE queue never
            # stalls on the Act exp stream.
            oT = otpool.tile([P, 2, N], BF16)
            with (
                tc.tile_pool(name="expool", bufs=3) as expool,
                tc.tile_pool(name="srpool", bufs=3) as srpool,
                tc.tile_pool(name="spsum", bufs=2, space="PSUM") as spsum,
                tc.tile_pool(name="opsum", bufs=4, space="PSUM") as opsum,
            ):
                units = [(h, qg) for h in range(HL) for qg in range(2)]
                LAG = 2
                state = {}  # unit idx -> (h, qg, ex, [pso0, pso1])

                def scores_step(i, kc):
                    h, qg, ex, _ = state[i]
                    hp = slice(32 * h, 32 * h + 32)
                    ps = spsum.tile([P, 1024], F32, tag="es")
                    for half in range(2):
                        q0 = qg * 1024 + half * 512
                        nc.tensor.matmul(
                            ps[:, half * 512 : (half + 1) * 512],
                            lhsT=kT8[hp, :, kc * P : (kc + 1) * P],
                            rhs=qT8[hp, :, q0 : q0 + 512],
                            start=True,
                            stop=True,
                            perf_mode=DR,
                            tile_position=(32 * h, 0),
                        )
                    nc.scalar.activation(
                        out=ex[:, kc, :], in_=ps[:], func=AF.Exp, scale=SCALE
                    )
                    nc.vector.tensor_mul(
                        out=ex[:, kc, :],
                        in0=ex[:, kc, :],
                        in1=mT[:, kc, qg * 1024 : (qg + 1) * 1024],
                    )

                def pv_step(i, kc):
                    h, qg, ex, psos = state[i]
                    for qbh in range(2):
                        nc.tensor.matmul(
                            psos[qbh][:],
                            lhsT=v_sb[:, kc, h, :],
                            rhs=ex[:, kc, qbh * 512 : (qbh + 1) * 512],
                            start=(kc == 0),
                            stop=(kc == NKC - 1),
                        )

                def unit_end(i):
                    h, qg, ex, psos = state.pop(i)
                    dc, po = h // 2, (h % 2) * HD
                    for qbh in range(2):
                        pso = psos[qbh]
                        s_rep = srpool.tile([HD, 512], F32, tag="srep")
                        nc.vector.reciprocal(out=s_rep[:], in_=pso[HD:, :])
                        o_tmp = srpool.tile([HD, 512], BF16, tag="otmp")
                        nc.vector.tensor_mul(
                            out=o_tmp[:], in0=pso[0:HD, :], in1=s_rep[:]
                        )
                        q0 = qg * 1024 + qbh * 512
                        nc.sync.dma_start(
                            out=oT[po : po + HD, dc, q0 : q0 + 512], in_=o_tmp[:]
                        )

                def start_unit(i):
                    h, qg = units[i]
                    ex = expool.tile([P, NKC, 1024], BF16, tag="ex")
                    psos = [
                        opsum.tile([2 * HD, 512], F32, tag="pvo", name=f"pvo{i}_{j}")
                        for j in range(2)
                    ]
                    state[i] = (h, qg, ex, psos)

                for i in range(len(units)):
                    start_unit(i)
                    for kc in range(NKC):
                        scores_step(i, kc)
                        if i >= LAG:
                            pv_step(i - LAG, kc)
                    if i >= LAG:
                        unit_end(i - LAG)
                # drain the last LAG units with dense PV chains
                for i in range(len(units) - LAG, len(units)):
                    for kc in range(NKC):
                        pv_step(i, kc)
                    unit_end(i)

            # ---------------- Phase C: output projection ----------------
            with (
                tc.tile_pool(name="cout", bufs=3) as cout,
                tc.tile_pool(name="cpsum", bufs=3, space="PSUM") as cpsum,
            ):
                for qc in range(NQC):
                    pss = cpsum.tile([P, D], F32, tag="co")
                    for cc in range(2):
                        for nh in range(2):
                            nc.tensor.matmul(
                                pss[:, nh * 512 : (nh + 1) * 512],
                                lhsT=oT[:, cc, qc * P : (qc + 1) * P],
                                rhs=wo_sb[:, cc, nh * 512 : (nh + 1) * 512],
                                start=(cc == 0),
                                stop=(cc == 1),
                            )
                    o_sb = cout.tile([P, D], F32, tag="osb")
                    nc.vector.tensor_copy(out=o_sb[:], in_=pss[:])
                    nc.sync.dma_start(out=OUT[qc * P : (qc + 1) * P, :], in_=o_sb[:])

    nc.finalize()
    return nc


_NC = None


def _get_nc():
    global _NC
    if _NC is None:
        _NC = build_nc()
    return _NC


def _dr_weight(W, cols):
    """[P, KOP, 2, DL] fp8 layout for the DoubleRow projection of W[cols,:].T.

    Output feature order: slot s (dc), partition p -> local feature
    h*64 + s*32 + (p%32) with h = p//32.
    """
    Wl = W[cols, :]  # [DL, D] rows = local features
    p_idx = np.arange(P)
    lf = (p_idx // 32) * 64 + (p_idx % 32)  # base feature per partition
    out = np.empty((P, KOP, 2, DL), np.float32)
    for s in range(2):
        # [D, 128] input-feature-major for this slot
        w_slot = Wl[lf + 32 * s, :].T  # [D, 128]
        out[:, :, :, s * P : (s + 1) * P] = w_slot.reshape(KOP, 2, P, P).transpose(
            2, 0, 1, 3
        )
    return np.ascontiguousarray(out).astype(ml_dtypes.float8_e4m3)


def _dr_bias(b, cols):
    bl = np.asarray(b, np.float32)[cols]
    p_idx = np.arange(P)
    lf = (p_idx // 32) * 64 + (p_idx % 32)
    out = np.stack([bl[lf], bl[lf + 32]], axis=1)  # [P, 2]
    return np.ascontiguousarray(out)


def make_in_maps(Q, K, mask, Wq, bq, Wk, bk, Wv, bv, Wo, bo):
    Q = np.asarray(Q, np.float32)
    K = np.asarray(K, np.float32)
    mask = np.asarray(mask)
    Wq = np.asarray(Wq, np.float32)
    Wk = np.asarray(Wk, np.float32)
    Wv = np.asarray(Wv, np.float32)
    Wo = np.asarray(Wo, np.float32)
    qt8 = [
        np.ascontiguousarray(Q[b].T).astype(ml_dtypes.float8_e4m3) for b in range(B)
    ]
    kt8 = [
        np.ascontiguousarray(K[b].T).astype(ml_dtypes.float8_e4m3) for b in range(B)
    ]
    kt = [np.ascontiguousarray(K[b].T).astype(ml_dtypes.bfloat16) for b in range(B)]
    mt = [
        np.ascontiguousarray(mask[b].T).astype(ml_dtypes.bfloat16) for b in range(B)
    ]
    in_maps = []
    for c in range(8):
        b, hg = divmod(c, 4)
        cols = slice(hg * DL, (hg + 1) * DL)
        in_maps.append(
            {
                "qt8": qt8[b],
                "kt8": kt8[b],
                "kt": kt[b],
                "mt": mt[b],
                "wq8": _dr_weight(Wq, cols),
                "wk8": _dr_weight(Wk, cols),
                "wvt": np.ascontiguousarray(Wv[cols, :].T).astype(ml_dtypes.bfloat16),
                "wot": np.ascontiguousarray(Wo[:, cols].T).astype(ml_dtypes.bfloat16),
                "bq": _dr_bias(bq, cols),
                "bk": _dr_bias(bk, cols),
                "bv": np.ascontiguousarray(np.asarray(bv, np.float32)[cols]),
            }
        )
    return in_maps


def assemble(results, bo):
    O = np.zeros((B, N, D), np.float32)
    for c in range(8):
        b = c // 4
        O[b] += results[c]["out"]
    O += np.asarray(bo, np.float32)[None, None, :]
    return O


def kernel(Q, K, mask, Wq, bq, Wk, bk, Wv, bv, Wo, bo):
    nc = _get_nc()
    in_maps = make_in_maps(Q, K, mask, Wq, bq, Wk, bk, Wv, bv, Wo, bo)
    res = run_bass_kernel_spmd(nc, in_maps, core_ids=list(range(8)))
    return assemble(res.results, bo)


# revision 11
# speedup vs baseline: 1.4486x; 1.1986x over previous
"""Trainium2 Bass kernel for nn_MHA_36584531427723.

Sharding: 8 cores = 2 batches x 4 head-groups (4 heads of 64 dims each per
core). Each core computes its batch's Q/K/V projections restricted to its
head-group's 256 output features, attention for its 4 heads, and a partial
output projection (its 256 rows of Wo^T). The host sums the 4 partials per
batch and adds bo.

v2 (vs baseline):
  - Q/K projections and QK^T scores run as fp8e4m3 DoubleRow matmuls
    (host ships Q^T/K^T and Wq/Wk in fp8; K^T additionally in bf16 for the
    V projection). End-to-end rel err ~5e-3 (numpy study), gate is 2e-2.
  - Q_^T/K_^T are stored [128p = 4 heads x 32 feats, 2 slots, 2048] fp8 so
    per-head DoubleRow slices are [32, 2, *] with tile_position (32h, 0).
  - Phase B software-pipelines one (head, qg) unit: scores of unit i+1
    issue before PV of unit i, so the PE never stalls on Act's exp stream.
  - Mask multiplies split DVE/Pool (Pool via scalar_tensor_tensor which is
    cheaper than tensor_tensor on GPSIMD); s_row copy on DVE, not Act.
  - softmax: max-subtraction dropped (|E/32| < ~0.6); eps dropped.
"""

import numpy as np
import ml_dtypes

import concourse.bacc as bacc
import concourse.bass as bass  # noqa: F401
import concourse.mybir as mybir
import concourse.tile as tile
from concourse.bass_utils import run_bass_kernel_spmd

B, N, D = 2, 2048, 1024
H = 16
HD = 64
HL = 4  # heads per core
DL = HL * HD  # 256 local features
P = 128
KO = D // P  # 8 contraction chunks for projections
KOP = KO // 2  # 4 DoubleRow pair-chunks
NKC = N // P  # 16 k-token chunks
NQC = N // P
NPAN = 4
PANW = N // NPAN  # 512-wide token panels in the projection phase
SCALE = 1.0 / 32.0  # 1/sqrt(DIM_V)

# elementwise work split (per (h, qg) unit of 16 kc tiles)
MASK_DVE = 10  # kc < MASK_DVE: mask multiply on DVE; rest on Pool

F32 = mybir.dt.float32
BF16 = mybir.dt.bfloat16
FP8 = mybir.dt.float8e4
AF = mybir.ActivationFunctionType
DR = mybir.MatmulPerfMode.DoubleRow
ALU = mybir.AluOpType


def build_nc():
    nc = bacc.Bacc(None, target_bir_lowering=False)
    QT8 = nc.dram_tensor("qt8", (D, N), FP8, kind="ExternalInput")
    KT = nc.dram_tensor("kt", (D, N), BF16, kind="ExternalInput")
    MT = nc.dram_tensor("mt", (N, N), BF16, kind="ExternalInput")
    WQ8 = nc.dram_tensor("wq8", (P, KOP, 2, DL), FP8, kind="ExternalInput")
    WK8 = nc.dram_tensor("wk8", (P, KOP, 2, DL), FP8, kind="ExternalInput")
    WVT = nc.dram_tensor("wvt", (D, DL), BF16, kind="ExternalInput")
    WOT = nc.dram_tensor("wot", (DL, D), BF16, kind="ExternalInput")
    BQ = nc.dram_tensor("bq", (P, 2), F32, kind="ExternalInput")
    BK = nc.dram_tensor("bk", (P, 2), F32, kind="ExternalInput")
    BV = nc.dram_tensor("bv", (DL,), F32, kind="ExternalInput")
    OUT = nc.dram_tensor("out", (N, D), BF16, kind="ExternalOutput")

    qt8_r = QT8[:].rearrange("(ko p) q -> p ko q", p=P)
    kt_r = KT[:].rearrange("(ko p) q -> p ko q", p=P)
    mt_r = MT[:].rearrange("(kc p) q -> p kc q", p=P)

    with tile.TileContext(nc) as tc:
        with (
            tc.tile_pool(name="persist", bufs=1) as persist,
            tc.tile_pool(name="otpool", bufs=1) as otpool,
        ):
            # --- persistent tiles ---
            mT = persist.tile([P, NKC, N], BF16)  # 64KB/part
            qT8 = persist.tile([P, 2, N], FP8, tag="qT8")  # 4KB
            kT8 = persist.tile([P, 2, N], FP8, tag="kT8")
            v_sb = persist.tile([P, NKC, HL, 2 * HD], BF16, tag="v")
            bq_sb = persist.tile([P, 2], F32, tag="bq")
            bk_sb = persist.tile([P, 2], F32, tag="bk")
            bv_rep = persist.tile([P, HL, HD], F32, tag="bv")
            wo_sb = persist.tile([P, 2, D], BF16, tag="wo")

            nc.sync.dma_start(out=bq_sb[:], in_=BQ[:])
            nc.sync.dma_start(out=bk_sb[:], in_=BK[:])
            nc.sync.dma_start(
                out=bv_rep[:],
                in_=BV[:].rearrange("(h d) -> h d", h=HL)[None].to_broadcast(
                    (P, HL, HD)
                ),
            )
            for cc in range(2):
                nc.sync.dma_start(
                    out=wo_sb[:, cc, :],
                    in_=WOT[:].rearrange("(cc p) n -> p cc n", p=P)[:, cc, :],
                )
            nc.vector.memset(v_sb[:, :, :, HD:], 1.0)

            # ---------------- Phase A: projections ----------------
            with (
                tc.tile_pool(name="wpool", bufs=1) as wpool,
                tc.tile_pool(name="panpool", bufs=2) as panpool,
                tc.tile_pool(name="pjpsum", bufs=4, space="PSUM") as pjpsum,
                tc.tile_pool(name="vpsum", bufs=4, space="PSUM") as vpsum,
            ):
                wq_sb = wpool.tile([P, KOP, 2, DL], FP8, tag="wq")
                wk_sb = wpool.tile([P, KOP, 2, DL], FP8, tag="wk")
                wv_sb = wpool.tile([P, KO, DL], BF16, tag="wv")
                nc.sync.dma_start(out=wq_sb[:], in_=WQ8[:])
                nc.sync.dma_start(out=wk_sb[:], in_=WK8[:])
                nc.sync.dma_start(
                    out=wv_sb[:], in_=WVT[:].rearrange("(ko p) m -> p ko m", p=P)
                )

                for pan in range(NPAN):
                    qs = slice(pan * PANW, (pan + 1) * PANW)
                    qt8_pan = panpool.tile([P, KO, PANW], FP8, tag="qt8_pan")
                    kt8_pan = panpool.tile([P, KO, PANW], FP8, tag="kt8_pan")
                    kt_pan = panpool.tile([P, KO, PANW], BF16, tag="kt_pan")
                    nc.sync.dma_start(out=kt_pan[:], in_=kt_r[:, :, qs])
                    nc.sync.dma_start(out=qt8_pan[:], in_=qt8_r[:, :, qs])
                    nc.vector.tensor_copy(out=kt8_pan[:], in_=kt_pan[:])

                    # K_^T then Q_^T via fp8 DoubleRow; bias fused in evict
                    for pan_in, w_sb, b_sb, dst in (
                        (kt8_pan, wk_sb, bk_sb, kT8),
                        (qt8_pan, wq_sb, bq_sb, qT8),
                    ):
                        for dc in range(2):
                            ps = pjpsum.tile([P, PANW], F32, tag="pj")
                            for j in range(KOP):
                                nc.tensor.matmul(
                                    ps[:],
                                    lhsT=w_sb[:, j, :, dc * P : (dc + 1) * P],
                                    rhs=pan_in[:, 2 * j : 2 * j + 2, :],
                                    start=(j == 0),
                                    stop=(j == KOP - 1),
                                    perf_mode=DR,
                                )
                            nc.scalar.activation(
                                out=dst[:, dc, qs],
                                in_=ps[:],
                                func=AF.Identity,
                                bias=b_sb[:, dc : dc + 1],
                                scale=1.0,
                            )

                    # V natural layout (token-on-partition), bias via DVE add
                    for t4 in range(PANW // P):
                        tci = pan * (PANW // P) + t4
                        psv = vpsum.tile([P, DL], F32, tag="pv")
                        for ko in range(KO):
                            nc.tensor.matmul(
                                psv[:],
                                lhsT=kt_pan[:, ko, t4 * P : (t4 + 1) * P],
                                rhs=wv_sb[:, ko, :],
                                start=(ko == 0),
                                stop=(ko == KO - 1),
                            )
                        nc.vector.tensor_add(
                            out=v_sb[:, tci, :, 0:HD],
                            in0=psv[:].rearrange("p (h d) -> p h d", h=HL),
                            in1=bv_rep[:],
                        )

                # mask load last so it fills DMA gaps during phase A
                for kc2 in range(NKC // 2):
                    nc.sync.dma_start(
                        out=mT[:, 2 * kc2 : 2 * kc2 + 2, :],
                        in_=mt_r[:, 2 * kc2 : 2 * kc2 + 2, :],
                    )

            # ---------------- Phase B: attention ----------------
            # Lag-2 software pipeline: unit i's scores/exp/mask stream is
            # interleaved per-kc with unit (i-2)'s PV accumulation steps whose
            # inputs are all long since ready, so the in-order PE queue never
            # stalls on the Act exp stream.
            oT = otpool.tile([P, 2, N], BF16)
            with (
                tc.tile_pool(name="expool", bufs=3) as expool,
                tc.tile_pool(name="srpool", bufs=2) as srpool,
                tc.tile_pool(name="spsum", bufs=2, space="PSUM") as spsum,
                tc.tile_pool(name="opsum", bufs=4, space="PSUM") as opsum,
            ):
                units = [(h, qg) for h in range(HL) for qg in range(2)]
                LAG = 2
                state = {}  # unit idx -> (h, qg, ex, [pso0, pso1])

                def scores_step(i, kc):
                    h, qg, ex, _ = state[i]
                    hp = slice(32 * h, 32 * h + 32)
                    ps = spsum.tile([P, 1024], F32, tag="es")
                    for half in range(2):
                        q0 = qg * 1024 + half * 512
                        nc.tensor.matmul(
                            ps[:, half * 512 : (half + 1) * 512],
                            lhsT=kT8[hp, :, kc * P : (kc + 1) * P],
                            rhs=qT8[hp, :, q0 : q0 + 512],
                            start=True,
                            stop=True,
                            perf_mode=DR,
                            tile_position=(32 * h, 0),
                        )
                    nc.scalar.activation(
                        out=ex[:, kc, :], in_=ps[:], func=AF.Exp, scale=SCALE
                    )
                    nc.vector.tensor_mul(
                        out=ex[:, kc, :],
                        in0=ex[:, kc, :],
                        in1=mT[:, kc, qg * 1024 : (qg + 1) * 1024],
                    )

                def pv_step(i, kc):
                    h, qg, ex, psos = state[i]
                    for qbh in range(2):
                        nc.tensor.matmul(
                            psos[qbh][:],
                            lhsT=v_sb[:, kc, h, :],
                            rhs=ex[:, kc, qbh * 512 : (qbh + 1) * 512],
                            start=(kc == 0),
                            stop=(kc == NKC - 1),
                        )

                def unit_end(i):
                    h, qg, ex, psos = state.pop(i)
                    dc, po = h // 2, (h % 2) * HD
                    o_tmp = srpool.tile([HD, 2, 512], BF16, tag="otmp")
                    for qbh in range(2):
                        pso = psos[qbh]
                        s_rep = srpool.tile([HD, 512], F32, tag="srep")
                        nc.vector.reciprocal(out=s_rep[:], in_=pso[HD:, :])
                        nc.vector.tensor_mul(
                            out=o_tmp[:, qbh, :], in0=pso[0:HD, :], in1=s_rep[:]
                        )
                    q0 = qg * 1024
                    nc.sync.dma_start(
                        out=oT[po : po + HD, dc, q0 : q0 + 1024],
                        in_=o_tmp[:].rearrange("p a b -> p (a b)"),
                    )

                def start_unit(i):
                    h, qg = units[i]
                    ex = expool.tile([P, NKC, 1024], BF16, tag="ex")
                    psos = [
                        opsum.tile([2 * HD, 512], F32, tag="pvo", name=f"pvo{i}_{j}")
                        for j in range(2)
                    ]
                    state[i] = (h, qg, ex, psos)

                for i in range(len(units)):
                    start_unit(i)
                    for kc in range(NKC):
                        scores_step(i, kc)
                        if i >= LAG:
                            pv_step(i - LAG, kc)
                    if i >= LAG:
                        unit_end(i - LAG)
                # drain the last LAG units with dense PV chains
                for i in range(len(units) - LAG, len(units)):
                    for kc in range(NKC):
                        pv_step(i, kc)
                    unit_end(i)

            # ---------------- Phase C: output projection ----------------
            with (
                tc.tile_pool(name="cout", bufs=3) as cout,
                tc.tile_pool(name="cpsum", bufs=3, space="PSUM") as cpsum,
            ):
                out_r = OUT[:].rearrange("(qc p) n -> p qc n", p=P)
                for qp in range(NQC // 2):
                    o_sb = cout.tile([P, 2, D], BF16, tag="osb")
                    for qh in range(2):
                        qc = 2 * qp + qh
                        pss = cpsum.tile([P, D], F32, tag="co")
                        for cc in range(2):
                            for nh in range(2):
                                nc.tensor.matmul(
                                    pss[:, nh * 512 : (nh + 1) * 512],
                                    lhsT=oT[:, cc, qc * P : (qc + 1) * P],
                                    rhs=wo_sb[:, cc, nh * 512 : (nh + 1) * 512],
                                    start=(cc == 0),
                                    stop=(cc == 1),
                                )
                        nc.vector.tensor_copy(out=o_sb[:, qh, :], in_=pss[:])
                    nc.sync.dma_start(
                        out=out_r[:, 2 * qp : 2 * qp + 2, :], in_=o_sb[:]
                    )

    nc.finalize()
    return nc


_NC = None


def _get_nc():
    global _NC
    if _NC is None:
        _NC = build_nc()
    return _NC


def _dr_weight(W, cols):
    """[P, KOP, 2, DL] fp8 layout for the DoubleRow projection of W[cols,:].T.

    Output feature order: slot s (dc), partition p -> local feature
    h*64 + s*32 + (p%32) with h = p//32.
    """
    Wl = W[cols, :]  # [DL, D] rows = local features
    p_idx = np.arange(P)
    lf = (p_idx // 32) * 64 + (p_idx % 32)  # base feature per partition
    out = np.empty((P, KOP, 2, DL), np.float32)
    for s in range(2):
        # [D, 128] input-feature-major for this slot
        w_slot = Wl[lf + 32 * s, :].T  # [D, 128]
        out[:, :, :, s * P : (s + 1) * P] = w_slot.reshape(KOP, 2, P, P).transpose(
            2, 0, 1, 3
        )
    return np.ascontiguousarray(out).astype(ml_dtypes.float8_e4m3)


def _dr_bias(b, cols):
    bl = np.asarray(b, np.float32)[cols]
    p_idx = np.arange(P)
    lf = (p_idx // 32) * 64 + (p_idx % 32)
    out = np.stack([bl[lf], bl[lf + 32]], axis=1)  # [P, 2]
    return np.ascontiguousarray(out)


def make_in_maps(Q, K, mask, Wq, bq, Wk, bk, Wv, bv, Wo, bo):
    Q = np.asarray(Q, np.float32)
    K = np.asarray(K, np.float32)
    mask = np.asarray(mask)
    Wq = np.asarray(Wq, np.float32)
    Wk = np.asarray(Wk, np.float32)
    Wv = np.asarray(Wv, np.float32)
    Wo = np.asarray(Wo, np.float32)
    qt8 = [
        np.ascontiguousarray(Q[b].T).astype(ml_dtypes.float8_e4m3) for b in range(B)
    ]
    kt = [np.ascontiguousarray(K[b].T).astype(ml_dtypes.bfloat16) for b in range(B)]
    mt = [
        np.ascontiguousarray(mask[b].T).astype(ml_dtypes.bfloat16) for b in range(B)
    ]
    in_maps = []
    for c in range(8):
        b, hg = divmod(c, 4)
        cols = slice(hg * DL, (hg + 1) * DL)
        in_maps.append(
            {
                "qt8": qt8[b],
                "kt": kt[b],
                "mt": mt[b],
                "wq8": _dr_weight(Wq, cols),
                "wk8": _dr_weight(Wk, cols),
                "wvt": np.ascontiguousarray(Wv[cols, :].T).astype(ml_dtypes.bfloat16),
                "wot": np.ascontiguousarray(Wo[:, cols].T).astype(ml_dtypes.bfloat16),
                "bq": _dr_bias(bq, cols),
                "bk": _dr_bias(bk, cols),
                "bv": np.ascontiguousarray(np.asarray(bv, np.float32)[cols]),
            }
        )
    return in_maps


def assemble(results, bo):
    O = np.zeros((B, N, D), np.float32)
    for c in range(8):
        b = c // 4
        O[b] += np.asarray(results[c]["out"], np.float32)
    O += np.asarray(bo, np.float32)[None, None, :]
    return O


def kernel(Q, K, mask, Wq, bq, Wk, bk, Wv, bv, Wo, bo):
    nc = _get_nc()
    in_maps = make_in_maps(Q, K, mask, Wq, bq, Wk, bk, Wv, bv, Wo, bo)
    res = run_bass_kernel_spmd(nc, in_maps, core_ids=list(range(8)))
    return assemble(res.results, bo)
